# revision 2
# baseline (speedup 1.0000x reference)
"""CapsuleLayer dynamic-routing kernel v2 for Trainium2 (Bass/Tile), SPMD x8.

Problem (full): x [256,1152,8], W [10,1152,8,16];
  priors[c,b,n,o] = sum_i x[b,n,i] W[c,n,i,o]; 3 routing iters;
  out [10,256,1,1,16].

Per core (b=32 local batch):
  Iter 0 (uniform probs): s0 = (1/N) sum_n priors via PE contraction over
  (n,i) chunks j=(i*9+nb), k=nl=128: stationary wk[nl,j,(c,o)], moving
  x(c'-replicated).  Caps 0-7 packed: out [128=(8c,o), 128=(c',b)] - the
  c' label is cosmetic at iter 0 (x identical for all c'), diag blocks by
  row-block give all 8 capsules.  Caps 8,9 separate [32,64].

  Iters 1,2 per group g in {0-3},{4-7},{8,9}:
    V^B[(c,b),(nb,nl,i)] = a^B(g)^T @ wt_g   (a stationary, PE)
    ACT evac psum->fp16, DVE xv = x^B * V (fp16 2x), DVE reduce_i -> l fp32
    rowmax/exp(l-max) with fused Z on ACT  (numerically safe softmax)
    e' = e^l * (1/Z) per-row (DVE tensor_scalar), PE-transpose e' -> [nl,...]
    m = x * e'^T (fp16), s = sum_j wk^T m (PE, groups 0-3/4-7 share one
    [128,256] matmul), squash (tiny PE broadcasts + DVE), a += out.
"""

import os
from contextlib import ExitStack

import numpy as np

B, N, CI, CO, NCAPS = 256, 1152, 8, 16, 10
NCORES = 8
BL = B // NCORES            # 32
NB = N // 128               # 9
NCH = CI * NB               # 72 chunks, j = i*9 + nb
GRP = ((0, 4), (4, 8), (8, 10))   # capsule groups
NIT = 3

_compiled = None


def _build():
    import concourse.bacc as bacc
    import concourse.mybir as mybir
    import concourse.tile as tile

    fp32 = mybir.dt.float32
    fp16 = mybir.dt.float16
    Alu = mybir.AluOpType
    Act = mybir.ActivationFunctionType
    AX = mybir.AxisListType

    nc = bacc.Bacc("TRN2", target_bir_lowering=False, debug=False)

    # ---- DRAM I/O (host-prearranged layouts; all contiguous copies) ----
    wk_d = nc.dram_tensor("wk", [128, NCH, NCAPS * CO], fp16,
                          kind="ExternalInput")
    # wt01: caps 0-3 rows 0-63, caps 4-7 rows 64-127 (partition-aligned with
    # the a01 slices used as stationary operands -> same PE row groups)
    wt01_d = nc.dram_tensor("wt01", [128, NB * 128 * CI], fp16,
                            kind="ExternalInput")
    wt2_d = nc.dram_tensor("wt2", [32, NB * 128 * CI], fp16,
                           kind="ExternalInput")
    xb_d = nc.dram_tensor("xb", [128, NB * 128 * CI], fp16,
                          kind="ExternalInput")
    xt4_d = nc.dram_tensor("xt4", [128, NCH, 128], fp16,
                           kind="ExternalInput")
    cf_d = nc.dram_tensor("cf", [128, 512], fp32, kind="ExternalInput")
    ch_d = nc.dram_tensor("ch", [128, 160], fp16, kind="ExternalInput")
    o1_d = nc.dram_tensor("o1", [128, 256], fp32, kind="ExternalOutput")
    o2_d = nc.dram_tensor("o2", [32, 64], fp32, kind="ExternalOutput")

    FNI = NB * 128 * CI      # 9216 free cols (nb, nl, i)

    with tile.TileContext(nc) as tc, ExitStack() as ctx:
        P = ctx.enter_context(tc.tile_pool(name="persist", bufs=1))
        vsb = ctx.enter_context(tc.tile_pool(name="vsb", bufs=2))
        xvp = ctx.enter_context(tc.tile_pool(name="xvp", bufs=2))
        lp = ctx.enter_context(tc.tile_pool(name="lp", bufs=1))
        ep = ctx.enter_context(tc.tile_pool(name="ep", bufs=1))
        etp = ctx.enter_context(tc.tile_pool(name="etp", bufs=1))
        mp = ctx.enter_context(tc.tile_pool(name="mp", bufs=2))
        smp = ctx.enter_context(tc.tile_pool(name="smp", bufs=1))
        abp = ctx.enter_context(tc.tile_pool(name="abp", bufs=2))
        ps_v = ctx.enter_context(tc.tile_pool(name="ps_v", bufs=2,
                                              space="PSUM"))
        ps_s = ctx.enter_context(tc.tile_pool(name="ps_s", bufs=1,
                                              space="PSUM"))
        ps_sm = ctx.enter_context(tc.tile_pool(name="ps_sm", bufs=1,
                                               space="PSUM"))

        # ---- persistent SBUF ----
        wk = P.tile([128, NCH, NCAPS * CO], fp16)    # [nl, j, (c,o)]
        wt01 = P.tile([128, FNI], fp16)              # [(c,o), (nb,nl,i)]
        wt2 = P.tile([32, FNI], fp16)
        xB = P.tile([128, FNI], fp16)                # [(c4,b), (nb,nl,i)]
        xt4 = P.tile([128, NCH, 128], fp16)          # [nl, j, (c'4, b)]
        cf = P.tile([128, 512], fp32)                # fp32 consts
        ch = P.tile([128, 160], fp16)                # fp16 consts (identity)
        mask01 = cf[:, 0:256]          # [128,256] diag mask for 8-cap concat
        mask2 = cf[0:32, 256:320]      # [32,64] diag mask caps 8,9
        bd8 = cf[:, 320:328]           # [128,8] sum_o selector (8 caps)
        bd2 = cf[0:32, 328:330]        # [32,2]
        sc8 = cf[0:8, 330:458]         # [8,128] c -> (c,o) broadcast
        sc2 = cf[0:2, 458:490]         # [2,32]
        ident = ch[:, 0:128]           # [128,128] fp16 identity

        # ---- input DMAs: it0-critical tensors (wk, xt4) stream in thirds
        # across all 3 queues; iter-1 tensors follow.
        nc.sync.dma_start(cf[:], cf_d[:])
        nc.sync.dma_start(ch[:], ch_d[:])
        qs = (nc.sync, nc.scalar, nc.gpsimd)
        for q in range(3):
            j0, j1 = q * 24, (q + 1) * 24
            qs[q].dma_start(wk[:, j0:j1, :], wk_d[:, j0:j1, :])
        for q in range(3):
            j0, j1 = q * 24, (q + 1) * 24
            qs[q].dma_start(xt4[:, j0:j1, :], xt4_d[:, j0:j1, :])
        nc.gpsimd.dma_start(wt01[:], wt01_d[:])
        nc.scalar.dma_start(xB[:], xb_d[:])
        nc.sync.dma_start(wt2[:], wt2_d[:])

        FB_g = (128, 128, 64)    # (c,b) rows/cols per group

        a01 = P.tile([128, 256], fp32)      # accumulated a, caps 0-7 concat
        a2 = P.tile([32, 64], fp32)         # caps 8,9
        aB = [None, None, None]             # fp16 block-diag a per group

        def squash(it, s01p, s2x):
            """s psum -> outputs/a-update.  s01p [128,256]; s2x [32,192]
            packs s2 (cols 0:64), snp2 (64:128), frp2 (128:192)."""
            zc = 1.0 / N if it == 0 else 1.0
            s2p = s2x[0:32, 0:64]
            ssb1 = smp.tile([128, 256], fp32, tag="ssb1")
            nc.scalar.copy(ssb1[:], s01p[:])
            ssb2 = smp.tile([32, 64], fp32, tag="ssb2")
            nc.scalar.copy(ssb2[:], s2p)
            s21 = smp.tile([128, 256], fp32, tag="s21")
            nc.vector.tensor_tensor(s21[:], ssb1[:], ssb1[:], Alu.mult)
            s22 = smp.tile([32, 64], fp32, tag="s22")
            nc.vector.tensor_tensor(s22[:], ssb2[:], ssb2[:], Alu.mult)
            sm1 = ps_sm.tile([128, 512], fp32, tag="sm1")
            snp1 = sm1[0:8, 256:512]
            nc.tensor.matmul(snp1, bd8, s21[:], start=True, stop=True)
            snp2 = s2x[0:2, 64:128]
            nc.tensor.matmul(snp2, bd2, s22[:], start=True, stop=True)
            # f = sqrt(sn)/(1+sn) (with uniform-probs 1/N^2 fold at iter 0)
            fs = []
            for snp, pn, fw in ((snp1, 8, 256), (snp2, 2, 64)):
                snt = smp.tile([pn, fw], fp32, tag=f"snt{pn}")
                if it == 0:
                    nc.vector.tensor_scalar_mul(snt[:], snp, zc * zc)
                else:
                    nc.scalar.copy(snt[:], snp)
                sq = smp.tile([pn, fw], fp32, tag=f"sq{pn}")
                nc.scalar.sqrt(sq[:], snt[:])
                den = smp.tile([pn, fw], fp32, tag=f"den{pn}")
                nc.vector.tensor_scalar_add(den[:], snt[:], 1.0)
                rden = smp.tile([pn, fw], fp32, tag=f"rden{pn}")
                nc.vector.reciprocal_approx_fast(rden[:], den[:])
                f = smp.tile([pn, fw], fp32, tag=f"f{pn}")
                if it == 0:
                    t = smp.tile([pn, fw], fp32, tag=f"ft{pn}")
                    nc.vector.tensor_tensor(t[:], sq[:], rden[:], Alu.mult)
                    nc.vector.tensor_scalar_mul(f[:], t[:], zc)
                else:
                    nc.vector.tensor_tensor(f[:], sq[:], rden[:], Alu.mult)
                fs.append(f)
            frp1 = sm1[:, 0:256]
            nc.tensor.matmul(frp1, sc8, fs[0][:], start=True, stop=True)
            frp2 = s2x[0:32, 128:192]
            nc.tensor.matmul(frp2, sc2, fs[1][:], start=True, stop=True)
            ov1 = smp.tile([128, 256], fp32, tag="ov1")
            nc.vector.tensor_tensor(ov1[:], ssb1[:], frp1, Alu.mult)
            ov2 = smp.tile([32, 64], fp32, tag="ov2")
            nc.vector.tensor_tensor(ov2[:], ssb2[:], frp2, Alu.mult)
            if it == NIT - 1:
                nc.sync.dma_start(o1_d[:], ov1[:])
                nc.sync.dma_start(o2_d[:], ov2[:])
                return
            if it == 0:
                nc.vector.tensor_tensor(a01[:], ov1[:], mask01, Alu.mult)
                nc.vector.tensor_tensor(a2[:], ov2[:], mask2, Alu.mult)
            else:
                t1 = smp.tile([128, 256], fp32, tag="au1")
                nc.vector.tensor_tensor(t1[:], a01[:], ov1[:], Alu.add)
                nc.vector.tensor_tensor(a01[:], t1[:], mask01, Alu.mult)
                t2 = smp.tile([32, 64], fp32, tag="au2")
                nc.vector.tensor_tensor(t2[:], a2[:], ov2[:], Alu.add)
                nc.vector.tensor_tensor(a2[:], t2[:], mask2, Alu.mult)
            # fp16 block-diag copies for the V-matmul stationary operand.
            # a01f16 keeps caps 4-7 on partitions 64-127, matching wt01.
            a01f = abp.tile([128, 256], fp16, tag="a01f")
            nc.scalar.copy(a01f[:], a01[:])
            a2f = abp.tile([32, 64], fp16, tag="a2f")
            nc.scalar.copy(a2f[:], a2[:])
            aB[0] = a01f[0:64, 0:128]
            aB[1] = a01f[64:128, 128:256]
            aB[2] = a2f[:]

        # ================= iteration 0: uniform probs =================
        s01p = ps_s.tile([128, 256], fp32, tag="s01")
        s2x = ps_s.tile([32, 192], fp32, tag="s2x")
        for j in range(NCH):
            nc.tensor.matmul(s01p[:, 0:128], wk[:, j, 0:128], xt4[:, j, :],
                             start=(j == 0), stop=(j == NCH - 1))
        for j in range(NCH):
            nc.tensor.matmul(s2x[0:32, 0:64], wk[:, j, 128:160],
                             xt4[:, j, 0:64],
                             start=(j == 0), stop=(j == NCH - 1))
        # mirror caps 0-7 block into cols 128-255 (c' cosmetic at iter 0)
        nc.scalar.copy(s01p[:, 128:256], s01p[:, 0:128])
        squash(0, s01p, s2x)

        # ================= iterations 1, 2 =================
        def wt_of(g):
            return (wt01[0:64, :], wt01[64:128, :], wt2[:])[g]

        for it in range(1, NIT):
            # ---- phase A: V = a^T @ wt + evac + xv + reduce, (nb, g)-interleaved
            lts = []
            for g in range(3):
                lt = lp.tile([128, NB, 128], fp16, tag=f"l{g}",
                             name=f"lt{g}")
                lts.append(lt)
            for nb in range(NB):
                for g in range(3):
                    fb = FB_g[g]
                    ag, wtg = aB[g], wt_of(g)
                    vp = ps_v.tile([128, 1024], fp32, tag="vp", name="vp")
                    c0 = nb * 1024
                    nc.tensor.matmul(vp[0:fb, 0:512], ag,
                                     wtg[:, c0:c0 + 512],
                                     start=True, stop=True)
                    nc.tensor.matmul(vp[0:fb, 512:1024], ag,
                                     wtg[:, c0 + 512:c0 + 1024],
                                     start=True, stop=True)
                    vs = vsb.tile([128, 1024], fp16, tag=f"vs{g}",
                                  name=f"vs{g}")
                    nc.scalar.copy(vs[0:fb, :], vp[0:fb, :])
                    # one DVE chunk per (nb,g) -> xv then i-sum via add tree
                    # (free order inside a chunk is (i, nl): halves = i-halves)
                    xv = xvp.tile([128, 1024], fp16, tag=f"xv{g}",
                                  name=f"xv{g}")
                    nc.vector.tensor_tensor(xv[0:fb, :],
                                            xB[0:fb, c0:c0 + 1024],
                                            vs[0:fb, :], Alu.mult)
                    t1 = xvp.tile([128, 512], fp16, tag=f"tr1{g}",
                                  name=f"tr1{g}")
                    nc.vector.tensor_tensor(t1[0:fb, :], xv[0:fb, 0:512],
                                            xv[0:fb, 512:1024], Alu.add)
                    t2 = xvp.tile([128, 256], fp16, tag=f"tr2{g}",
                                  name=f"tr2{g}")
                    nc.vector.tensor_tensor(t2[0:fb, :], t1[0:fb, 0:256],
                                            t1[0:fb, 256:512], Alu.add)
                    nc.vector.tensor_tensor(lts[g][0:fb, nb, :],
                                            t2[0:fb, 0:128],
                                            t2[0:fb, 128:256], Alu.add)
            # ---- phase B: safe softmax over n per group (n is free)
            ees = []
            for g in range(3):
                fb = FB_g[g]
                lv = lts[g][0:fb, :, :]
                mx = smp.tile([128, 1], fp16, tag=f"mx{g}", name=f"mx{g}")
                with nc.allow_low_precision(reason="rowmax fp16"):
                    nc.vector.tensor_reduce(mx[0:fb, :], lv, AX.XY, Alu.max)
                nmx = smp.tile([128, 1], fp32, tag=f"nmx{g}", name=f"nmx{g}")
                nc.vector.tensor_scalar_mul(nmx[0:fb, :], mx[0:fb, :], -1.0)
                el = ep.tile([128, NB, 128], fp16, tag=f"el{g}",
                             name=f"el{g}")
                zz = smp.tile([128, 1], fp32, tag=f"zz{g}", name=f"zz{g}")
                nc.scalar.activation(el[0:fb, :, :], lv, Act.Exp,
                                     bias=nmx[0:fb, :],
                                     accum_out=zz[0:fb, :])
                rz = smp.tile([128, 1], fp32, tag=f"rz{g}", name=f"rz{g}")
                nc.vector.reciprocal_approx_fast(rz[0:fb, :], zz[0:fb, :])
                ee = ep.tile([128, NB, 128], fp16, tag=f"ee{g}",
                             name=f"ee{g}")
                nc.vector.tensor_scalar_mul(ee[0:fb, :, :], el[0:fb, :, :],
                                            rz[0:fb, :])
                ees.append(ee)
            # ---- phase C: transpose e' -> [nl, nb, (c,b)] via PE
            eT = []
            for g in range(3):
                fb = FB_g[g]
                etpp = ps_v.tile([128, NB, 128], fp16, tag="vp", name="etpp")
                for nb in range(NB):
                    nc.tensor.matmul(etpp[:, nb, 0:fb], ees[g][0:fb, nb, :],
                                     ident[0:fb, 0:fb], is_transpose=True,
                                     start=True, stop=True)
                et = etp.tile([128, NB, fb], fp16, tag=f"et{g}",
                              name=f"et{g}")
                nc.scalar.copy(et[:], etpp[:, :, 0:fb])
                eT.append(et)
            # ---- phase D: m = x * e'^T (m2 on gpsimd) ; s = sum_j wk^T m
            s01p = ps_s.tile([128, 256], fp32, tag="s01")
            s2x = ps_s.tile([32, 192], fp32, tag="s2x")
            for i in range(CI):
                m01 = mp.tile([128, NB, 256], fp16, tag="m01")
                j0 = i * NB
                nc.vector.tensor_tensor(m01[:, :, 0:128],
                                        xt4[:, j0:j0 + NB, :], eT[0][:],
                                        Alu.mult)
                nc.vector.tensor_tensor(m01[:, :, 128:256],
                                        xt4[:, j0:j0 + NB, :], eT[1][:],
                                        Alu.mult)
                m2 = mp.tile([128, NB, 64], fp16, tag="m2")
                nc.vector.tensor_tensor(m2[:], xt4[:, j0:j0 + NB, 0:64],
                                        eT[2][:], Alu.mult)
                for nb in range(NB):
                    j = j0 + nb
                    nc.tensor.matmul(s01p[:], wk[:, j, 0:128], m01[:, nb, :],
                                     start=(j == 0), stop=(j == NCH - 1))
                    nc.tensor.matmul(s2x[0:32, 0:64], wk[:, j, 128:160],
                                     m2[:, nb, :],
                                     start=(j == 0), stop=(j == NCH - 1))
            squash(it, s01p, s2x)

    nc.compile()
    return nc


def _get_compiled():
    global _compiled
    if _compiled is None:
        _compiled = _build()
    return _compiled


def _make_consts():
    cf = np.zeros((128, 512), dtype=np.float32)
    # mask01 [128,256]: row (q,o) q in 0..7; col block c' in 0..3 per half
    for q in range(8):
        cp = q % 4
        half = (q // 4) * 128
        cf[q * CO:(q + 1) * CO, half + cp * BL:half + (cp + 1) * BL] = 1.0
        cf[q * CO:(q + 1) * CO, 320 + q] = 1.0                  # bd8
        cf[q, 330 + q * CO:330 + (q + 1) * CO] = 1.0            # sc8 [8,128]
    for q in range(2):
        cf[q * CO:(q + 1) * CO, 256 + q * BL:256 + (q + 1) * BL] = 1.0  # mask2
        cf[q * CO:(q + 1) * CO, 328 + q] = 1.0                  # bd2
        cf[q, 458 + q * CO:458 + (q + 1) * CO] = 1.0            # sc2 [2,32]
    ch = np.zeros((128, 160), dtype=np.float16)
    ch[:, 0:128] = np.eye(128, dtype=np.float16)
    return cf, ch


def _prep_w(route_weights):
    w5 = np.ascontiguousarray(route_weights, dtype=np.float32).reshape(
        NCAPS, NB, 128, CI, CO)
    wk = np.ascontiguousarray(
        w5.transpose(2, 3, 1, 0, 4).reshape(128, NCH, NCAPS * CO)
    ).astype(np.float16)
    wt01 = np.ascontiguousarray(
        w5[0:8].transpose(0, 4, 1, 3, 2)            # c,o,nb,i,nl
        .reshape(128, NB * 128 * CI)).astype(np.float16)
    wt2 = np.ascontiguousarray(
        w5[8:10].transpose(0, 4, 1, 3, 2)
        .reshape(32, NB * 128 * CI)).astype(np.float16)
    return wk, wt01, wt2


def _prep_x_shard(xs):
    # xt4 [nl, j=(i*9+nb), (c'4, b)]
    x4 = xs.reshape(BL, NB, 128, CI)
    xt = np.ascontiguousarray(
        x4.transpose(2, 3, 1, 0).reshape(128, NCH, BL)).astype(np.float16)
    xt4 = np.ascontiguousarray(np.tile(xt, (1, 1, 4)))
    # xB [(c4,b), (nb, i, nl)]
    xb0 = np.ascontiguousarray(
        x4.transpose(0, 1, 3, 2).reshape(BL, -1))
    xb = np.ascontiguousarray(np.tile(xb0, (4, 1))).astype(np.float16)
    return xt4, xb


def _extract(o1, o2):
    out = np.empty((NCAPS, BL, CO), dtype=np.float32)
    for c in range(8):
        half, cl = (c // 4) * 128, c % 4
        out[c] = o1[c * CO:(c + 1) * CO,
                    half + cl * BL:half + (cl + 1) * BL].T
    for c in range(2):
        out[8 + c] = o2[c * CO:(c + 1) * CO, c * BL:(c + 1) * BL].T
    return out


def kernel(x: np.ndarray, route_weights: np.ndarray) -> np.ndarray:
    from concourse.bass_utils import run_bass_kernel_spmd

    nc = _get_compiled()
    x = np.ascontiguousarray(x, dtype=np.float32)
    wk, wt01, wt2 = _prep_w(route_weights)
    cf, ch = _make_consts()
    in_maps = []
    for ci in range(NCORES):
        xt4, xb = _prep_x_shard(x[ci * BL:(ci + 1) * BL])
        in_maps.append({"wk": wk, "wt01": wt01, "wt2": wt2,
                        "xb": xb, "xt4": xt4, "cf": cf, "ch": ch})
    res = run_bass_kernel_spmd(
        nc, in_maps, list(range(NCORES)),
        trace=bool(int(os.environ.get("CAPS_TRACE", "0"))))
    outs = [_extract(res.results[ci]["o1"], res.results[ci]["o2"])
            for ci in range(NCORES)]
    full = np.concatenate(outs, axis=1)
    if res.exec_time_ns is not None:
        kernel.last_exec_time_ns = res.exec_time_ns
    return full[:, :, None, None, :].astype(np.float32)


kernel.last_exec_time_ns = None


# revision 3
# speedup vs baseline: 1.0084x; 1.0084x over previous
"""CapsuleLayer dynamic-routing kernel v2 for Trainium2 (Bass/Tile), SPMD x8.

Problem (full): x [256,1152,8], W [10,1152,8,16];
  priors[c,b,n,o] = sum_i x[b,n,i] W[c,n,i,o]; 3 routing iters;
  out [10,256,1,1,16].

Per core (b=32 local batch):
  Iter 0 (uniform probs): s0 = (1/N) sum_n priors via PE contraction over
  (n,i) chunks j=(i*9+nb), k=nl=128: stationary wk[nl,j,(c,o)], moving
  x(c'-replicated).  Caps 0-7 packed: out [128=(8c,o), 128=(c',b)] - the
  c' label is cosmetic at iter 0 (x identical for all c'), diag blocks by
  row-block give all 8 capsules.  Caps 8,9 separate [32,64].

  Iters 1,2 per group g in {0-3},{4-7},{8,9}:
    V^B[(c,b),(nb,nl,i)] = a^B(g)^T @ wt_g   (a stationary, PE)
    ACT evac psum->fp16, DVE xv = x^B * V (fp16 2x), DVE reduce_i -> l fp32
    rowmax/exp(l-max) with fused Z on ACT  (numerically safe softmax)
    e' = e^l * (1/Z) per-row (DVE tensor_scalar), PE-transpose e' -> [nl,...]
    m = x * e'^T (fp16), s = sum_j wk^T m (PE, groups 0-3/4-7 share one
    [128,256] matmul), squash (tiny PE broadcasts + DVE), a += out.
"""

import os
from contextlib import ExitStack

import numpy as np

B, N, CI, CO, NCAPS = 256, 1152, 8, 16, 10
NCORES = 8
BL = B // NCORES            # 32
NB = N // 128               # 9
NCH = CI * NB               # 72 chunks, j = i*9 + nb
GRP = ((0, 4), (4, 8), (8, 10))   # capsule groups
NIT = 3

_compiled = None


def _build():
    import concourse.bacc as bacc
    import concourse.mybir as mybir
    import concourse.tile as tile

    fp32 = mybir.dt.float32
    fp16 = mybir.dt.float16
    Alu = mybir.AluOpType
    Act = mybir.ActivationFunctionType
    AX = mybir.AxisListType

    nc = bacc.Bacc("TRN2", target_bir_lowering=False, debug=False)

    # ---- DRAM I/O (host-prearranged layouts; all contiguous copies) ----
    wk_d = nc.dram_tensor("wk", [128, NCH, NCAPS * CO], fp16,
                          kind="ExternalInput")
    # wt01: caps 0-3 rows 0-63, caps 4-7 rows 64-127 (partition-aligned with
    # the a01 slices used as stationary operands -> same PE row groups)
    wt01_d = nc.dram_tensor("wt01", [128, NB * 128 * CI], fp16,
                            kind="ExternalInput")
    wt2_d = nc.dram_tensor("wt2", [32, NB * 128 * CI], fp16,
                           kind="ExternalInput")
    xb_d = nc.dram_tensor("xb", [128, NB * 128 * CI], fp16,
                          kind="ExternalInput")
    xt4_d = nc.dram_tensor("xt4", [128, NCH, 128], fp16,
                           kind="ExternalInput")
    cf_d = nc.dram_tensor("cf", [128, 512], fp32, kind="ExternalInput")
    ch_d = nc.dram_tensor("ch", [128, 160], fp16, kind="ExternalInput")
    o1_d = nc.dram_tensor("o1", [128, 256], fp32, kind="ExternalOutput")
    o2_d = nc.dram_tensor("o2", [32, 64], fp32, kind="ExternalOutput")

    FNI = NB * 128 * CI      # 9216 free cols (nb, nl, i)

    with tile.TileContext(nc) as tc, ExitStack() as ctx:
        P = ctx.enter_context(tc.tile_pool(name="persist", bufs=1))
        vsb = ctx.enter_context(tc.tile_pool(name="vsb", bufs=2))
        xvp = ctx.enter_context(tc.tile_pool(name="xvp", bufs=2))
        lp = ctx.enter_context(tc.tile_pool(name="lp", bufs=1))
        ep = ctx.enter_context(tc.tile_pool(name="ep", bufs=1))
        etp = ctx.enter_context(tc.tile_pool(name="etp", bufs=1))
        mp = ctx.enter_context(tc.tile_pool(name="mp", bufs=2))
        smp = ctx.enter_context(tc.tile_pool(name="smp", bufs=1))
        abp = ctx.enter_context(tc.tile_pool(name="abp", bufs=2))
        ps_v = ctx.enter_context(tc.tile_pool(name="ps_v", bufs=2,
                                              space="PSUM"))
        ps_s = ctx.enter_context(tc.tile_pool(name="ps_s", bufs=1,
                                              space="PSUM"))
        ps_sm = ctx.enter_context(tc.tile_pool(name="ps_sm", bufs=1,
                                               space="PSUM"))

        # ---- persistent SBUF ----
        wk = P.tile([128, NCH, NCAPS * CO], fp16)    # [nl, j, (c,o)]
        wt01 = P.tile([128, FNI], fp16)              # [(c,o), (nb,nl,i)]
        wt2 = P.tile([32, FNI], fp16)
        xB = P.tile([128, FNI], fp16)                # [(c4,b), (nb,nl,i)]
        xt4 = P.tile([128, NCH, 128], fp16)          # [nl, j, (c'4, b)]
        cf = P.tile([128, 512], fp32)                # fp32 consts
        ch = P.tile([128, 160], fp16)                # fp16 consts (identity)
        mask01 = cf[:, 0:256]          # [128,256] diag mask for 8-cap concat
        mask2 = cf[0:32, 256:320]      # [32,64] diag mask caps 8,9
        bd8 = cf[:, 320:328]           # [128,8] sum_o selector (8 caps)
        bd2 = cf[0:32, 328:330]        # [32,2]
        sc8 = cf[0:8, 330:458]         # [8,128] c -> (c,o) broadcast
        sc2 = cf[0:2, 458:490]         # [2,32]
        ident = ch[:, 0:128]           # [128,128] fp16 identity

        # ---- input DMAs: it0-critical tensors (wk, xt4) stream in thirds
        # across all 3 queues; iter-1 tensors follow.
        nc.sync.dma_start(cf[:], cf_d[:])
        nc.sync.dma_start(ch[:], ch_d[:])
        qs = (nc.sync, nc.scalar, nc.gpsimd)
        for q in range(3):
            j0, j1 = q * 24, (q + 1) * 24
            qs[q].dma_start(wk[:, j0:j1, :], wk_d[:, j0:j1, :])
        for q in range(3):
            j0, j1 = q * 24, (q + 1) * 24
            qs[q].dma_start(xt4[:, j0:j1, :], xt4_d[:, j0:j1, :])
        nc.gpsimd.dma_start(wt01[:], wt01_d[:])
        nc.scalar.dma_start(xB[:], xb_d[:])
        nc.sync.dma_start(wt2[:], wt2_d[:])

        FB_g = (128, 128, 64)    # (c,b) rows/cols per group

        a01 = P.tile([128, 256], fp32)      # accumulated a, caps 0-7 concat
        a2 = P.tile([32, 64], fp32)         # caps 8,9
        aB = [None, None, None]             # fp16 block-diag a per group

        def squash(it, s01p, s2x):
            """s psum -> outputs/a-update.  s01p [128,256]; s2x [32,192]
            packs s2 (cols 0:64), snp2 (64:128), frp2 (128:192)."""
            zc = 1.0 / N if it == 0 else 1.0
            s2p = s2x[0:32, 0:64]
            ssb1 = smp.tile([128, 256], fp32, tag="ssb1")
            nc.scalar.copy(ssb1[:], s01p[:])
            ssb2 = smp.tile([32, 64], fp32, tag="ssb2")
            nc.scalar.copy(ssb2[:], s2p)
            s21 = smp.tile([128, 256], fp32, tag="s21")
            nc.vector.tensor_tensor(s21[:], ssb1[:], ssb1[:], Alu.mult)
            s22 = smp.tile([32, 64], fp32, tag="s22")
            nc.vector.tensor_tensor(s22[:], ssb2[:], ssb2[:], Alu.mult)
            sm1 = ps_sm.tile([128, 512], fp32, tag="sm1")
            snp1 = sm1[0:8, 256:512]
            nc.tensor.matmul(snp1, bd8, s21[:], start=True, stop=True)
            snp2 = s2x[0:2, 64:128]
            nc.tensor.matmul(snp2, bd2, s22[:], start=True, stop=True)
            # f = sqrt(sn)/(1+sn) (with uniform-probs 1/N^2 fold at iter 0)
            fs = []
            for snp, pn, fw in ((snp1, 8, 256), (snp2, 2, 64)):
                snt = smp.tile([pn, fw], fp32, tag=f"snt{pn}")
                if it == 0:
                    nc.vector.tensor_scalar_mul(snt[:], snp, zc * zc)
                else:
                    nc.scalar.copy(snt[:], snp)
                sq = smp.tile([pn, fw], fp32, tag=f"sq{pn}")
                nc.scalar.sqrt(sq[:], snt[:])
                den = smp.tile([pn, fw], fp32, tag=f"den{pn}")
                nc.vector.tensor_scalar_add(den[:], snt[:], 1.0)
                rden = smp.tile([pn, fw], fp32, tag=f"rden{pn}")
                nc.vector.reciprocal_approx_fast(rden[:], den[:])
                f = smp.tile([pn, fw], fp32, tag=f"f{pn}")
                if it == 0:
                    t = smp.tile([pn, fw], fp32, tag=f"ft{pn}")
                    nc.vector.tensor_tensor(t[:], sq[:], rden[:], Alu.mult)
                    nc.vector.tensor_scalar_mul(f[:], t[:], zc)
                else:
                    nc.vector.tensor_tensor(f[:], sq[:], rden[:], Alu.mult)
                fs.append(f)
            frp1 = sm1[:, 0:256]
            nc.tensor.matmul(frp1, sc8, fs[0][:], start=True, stop=True)
            frp2 = s2x[0:32, 128:192]
            nc.tensor.matmul(frp2, sc2, fs[1][:], start=True, stop=True)
            ov1 = smp.tile([128, 256], fp32, tag="ov1")
            nc.vector.tensor_tensor(ov1[:], ssb1[:], frp1, Alu.mult)
            ov2 = smp.tile([32, 64], fp32, tag="ov2")
            nc.vector.tensor_tensor(ov2[:], ssb2[:], frp2, Alu.mult)
            if it == NIT - 1:
                nc.sync.dma_start(o1_d[:], ov1[:])
                nc.sync.dma_start(o2_d[:], ov2[:])
                return
            if it == 0:
                nc.vector.tensor_tensor(a01[:], ov1[:], mask01, Alu.mult)
                nc.vector.tensor_tensor(a2[:], ov2[:], mask2, Alu.mult)
            else:
                t1 = smp.tile([128, 256], fp32, tag="au1")
                nc.vector.tensor_tensor(t1[:], a01[:], ov1[:], Alu.add)
                nc.vector.tensor_tensor(a01[:], t1[:], mask01, Alu.mult)
                t2 = smp.tile([32, 64], fp32, tag="au2")
                nc.vector.tensor_tensor(t2[:], a2[:], ov2[:], Alu.add)
                nc.vector.tensor_tensor(a2[:], t2[:], mask2, Alu.mult)
            # fp16 block-diag copies for the V-matmul stationary operand.
            # a01f16 keeps caps 4-7 on partitions 64-127, matching wt01.
            a01f = abp.tile([128, 256], fp16, tag="a01f")
            nc.scalar.copy(a01f[:], a01[:])
            a2f = abp.tile([32, 64], fp16, tag="a2f")
            nc.scalar.copy(a2f[:], a2[:])
            aB[0] = a01f[0:64, 0:128]
            aB[1] = a01f[64:128, 128:256]
            aB[2] = a2f[:]

        # ================= iteration 0: uniform probs =================
        s01p = ps_s.tile([128, 256], fp32, tag="s01")
        s2x = ps_s.tile([32, 192], fp32, tag="s2x")
        for j in range(NCH):
            nc.tensor.matmul(s01p[:, 0:128], wk[:, j, 0:128], xt4[:, j, :],
                             start=(j == 0), stop=(j == NCH - 1))
        for j in range(NCH):
            nc.tensor.matmul(s2x[0:32, 0:64], wk[:, j, 128:160],
                             xt4[:, j, 0:64],
                             start=(j == 0), stop=(j == NCH - 1))
        # mirror caps 0-7 block into cols 128-255 (c' cosmetic at iter 0)
        nc.scalar.copy(s01p[:, 128:256], s01p[:, 0:128])
        squash(0, s01p, s2x)

        # ================= iterations 1, 2 =================
        def wt_of(g):
            return (wt01[0:64, :], wt01[64:128, :], wt2[:])[g]

        for it in range(1, NIT):
            # ---- phase A: V = a^T @ wt + evac + merged xv/i-sum tree.
            # All 3 groups' chunks pack into [128, 3, 1024] so each DVE op
            # covers every group at once (per-partition timing; g2's unused
            # rows ride free).  Free order in a chunk is (i, nl).
            l3 = lp.tile([128, NB, 3, 128], fp16, tag="l3", name="l3")
            for nb in range(NB):
                c0 = nb * 1024
                vs3 = vsb.tile([128, 3, 1024], fp16, tag="vs3", name="vs3")
                for g in range(3):
                    fb = FB_g[g]
                    ag, wtg = aB[g], wt_of(g)
                    vp = ps_v.tile([128, 1024], fp32, tag="vp", name="vp")
                    nc.tensor.matmul(vp[0:fb, 0:512], ag,
                                     wtg[:, c0:c0 + 512],
                                     start=True, stop=True)
                    nc.tensor.matmul(vp[0:fb, 512:1024], ag,
                                     wtg[:, c0 + 512:c0 + 1024],
                                     start=True, stop=True)
                    nc.scalar.copy(vs3[0:fb, g, :], vp[0:fb, :])
                xv3 = xvp.tile([128, 3, 1024], fp16, tag="xv3", name="xv3")
                for g in range(3):
                    fb = FB_g[g]
                    nc.vector.tensor_tensor(xv3[0:fb, g, :],
                                            xB[0:fb, c0:c0 + 1024],
                                            vs3[0:fb, g, :], Alu.mult)
                t1 = xvp.tile([128, 3, 512], fp16, tag="t1", name="t1")
                nc.vector.tensor_tensor(t1[:], xv3[:, :, 0:512],
                                        xv3[:, :, 512:1024], Alu.add)
                t2 = xvp.tile([128, 3, 256], fp16, tag="t2", name="t2")
                nc.vector.tensor_tensor(t2[:], t1[:, :, 0:256],
                                        t1[:, :, 256:512], Alu.add)
                nc.vector.tensor_tensor(l3[:, nb, :, :], t2[:, :, 0:128],
                                        t2[:, :, 128:256], Alu.add)
            # ---- phase B: safe softmax over n per group (n is free)
            ees = []
            for g in range(3):
                fb = FB_g[g]
                lv = l3[0:fb, :, g, :]
                mx = smp.tile([128, 1], fp16, tag=f"mx{g}", name=f"mx{g}")
                with nc.allow_low_precision(reason="rowmax fp16"):
                    nc.vector.tensor_reduce(mx[0:fb, :], lv, AX.XY, Alu.max)
                nmx = smp.tile([128, 1], fp32, tag=f"nmx{g}", name=f"nmx{g}")
                nc.vector.tensor_scalar_mul(nmx[0:fb, :], mx[0:fb, :], -1.0)
                el = ep.tile([128, NB, 128], fp16, tag=f"el{g}",
                             name=f"el{g}")
                zz = smp.tile([128, 1], fp32, tag=f"zz{g}", name=f"zz{g}")
                nc.scalar.activation(el[0:fb, :, :], lv, Act.Exp,
                                     bias=nmx[0:fb, :],
                                     accum_out=zz[0:fb, :])
                rz = smp.tile([128, 1], fp32, tag=f"rz{g}", name=f"rz{g}")
                nc.vector.reciprocal_approx_fast(rz[0:fb, :], zz[0:fb, :])
                ee = ep.tile([128, NB, 128], fp16, tag=f"ee{g}",
                             name=f"ee{g}")
                nc.vector.tensor_scalar_mul(ee[0:fb, :, :], el[0:fb, :, :],
                                            rz[0:fb, :])
                ees.append(ee)
            # ---- phase C: transpose e' -> [nl, nb, (c,b)] via PE
            eT = []
            for g in range(3):
                fb = FB_g[g]
                etpp = ps_v.tile([128, NB, 128], fp16, tag="vp", name="etpp")
                for nb in range(NB):
                    nc.tensor.matmul(etpp[:, nb, 0:fb], ees[g][0:fb, nb, :],
                                     ident[0:fb, 0:fb], is_transpose=True,
                                     start=True, stop=True)
                et = etp.tile([128, NB, fb], fp16, tag=f"et{g}",
                              name=f"et{g}")
                nc.scalar.copy(et[:], etpp[:, :, 0:fb])
                eT.append(et)
            # ---- phase D: m = x * e'^T (m2 on gpsimd) ; s = sum_j wk^T m
            s01p = ps_s.tile([128, 256], fp32, tag="s01")
            s2x = ps_s.tile([32, 192], fp32, tag="s2x")
            for i in range(CI):
                m01 = mp.tile([128, NB, 256], fp16, tag="m01")
                j0 = i * NB
                nc.vector.tensor_tensor(m01[:, :, 0:128],
                                        xt4[:, j0:j0 + NB, :], eT[0][:],
                                        Alu.mult)
                nc.vector.tensor_tensor(m01[:, :, 128:256],
                                        xt4[:, j0:j0 + NB, :], eT[1][:],
                                        Alu.mult)
                m2 = mp.tile([128, NB, 64], fp16, tag="m2")
                nc.vector.tensor_tensor(m2[:], xt4[:, j0:j0 + NB, 0:64],
                                        eT[2][:], Alu.mult)
                for nb in range(NB):
                    j = j0 + nb
                    nc.tensor.matmul(s01p[:], wk[:, j, 0:128], m01[:, nb, :],
                                     start=(j == 0), stop=(j == NCH - 1))
                    nc.tensor.matmul(s2x[0:32, 0:64], wk[:, j, 128:160],
                                     m2[:, nb, :],
                                     start=(j == 0), stop=(j == NCH - 1))
            squash(it, s01p, s2x)

    nc.compile()
    return nc


def _get_compiled():
    global _compiled
    if _compiled is None:
        _compiled = _build()
    return _compiled


def _make_consts():
    cf = np.zeros((128, 512), dtype=np.float32)
    # mask01 [128,256]: row (q,o) q in 0..7; col block c' in 0..3 per half
    for q in range(8):
        cp = q % 4
        half = (q // 4) * 128
        cf[q * CO:(q + 1) * CO, half + cp * BL:half + (cp + 1) * BL] = 1.0
        cf[q * CO:(q + 1) * CO, 320 + q] = 1.0                  # bd8
        cf[q, 330 + q * CO:330 + (q + 1) * CO] = 1.0            # sc8 [8,128]
    for q in range(2):
        cf[q * CO:(q + 1) * CO, 256 + q * BL:256 + (q + 1) * BL] = 1.0  # mask2
        cf[q * CO:(q + 1) * CO, 328 + q] = 1.0                  # bd2
        cf[q, 458 + q * CO:458 + (q + 1) * CO] = 1.0            # sc2 [2,32]
    ch = np.zeros((128, 160), dtype=np.float16)
    ch[:, 0:128] = np.eye(128, dtype=np.float16)
    return cf, ch


def _prep_w(route_weights):
    w5 = np.ascontiguousarray(route_weights, dtype=np.float32).reshape(
        NCAPS, NB, 128, CI, CO)
    wk = np.ascontiguousarray(
        w5.transpose(2, 3, 1, 0, 4).reshape(128, NCH, NCAPS * CO)
    ).astype(np.float16)
    wt01 = np.ascontiguousarray(
        w5[0:8].transpose(0, 4, 1, 3, 2)            # c,o,nb,i,nl
        .reshape(128, NB * 128 * CI)).astype(np.float16)
    wt2 = np.ascontiguousarray(
        w5[8:10].transpose(0, 4, 1, 3, 2)
        .reshape(32, NB * 128 * CI)).astype(np.float16)
    return wk, wt01, wt2


def _prep_x_shard(xs):
    # xt4 [nl, j=(i*9+nb), (c'4, b)]
    x4 = xs.reshape(BL, NB, 128, CI)
    xt = np.ascontiguousarray(
        x4.transpose(2, 3, 1, 0).reshape(128, NCH, BL)).astype(np.float16)
    xt4 = np.ascontiguousarray(np.tile(xt, (1, 1, 4)))
    # xB [(c4,b), (nb, i, nl)]
    xb0 = np.ascontiguousarray(
        x4.transpose(0, 1, 3, 2).reshape(BL, -1))
    xb = np.ascontiguousarray(np.tile(xb0, (4, 1))).astype(np.float16)
    return xt4, xb


def _extract(o1, o2):
    out = np.empty((NCAPS, BL, CO), dtype=np.float32)
    for c in range(8):
        half, cl = (c // 4) * 128, c % 4
        out[c] = o1[c * CO:(c + 1) * CO,
                    half + cl * BL:half + (cl + 1) * BL].T
    for c in range(2):
        out[8 + c] = o2[c * CO:(c + 1) * CO, c * BL:(c + 1) * BL].T
    return out


def kernel(x: np.ndarray, route_weights: np.ndarray) -> np.ndarray:
    from concourse.bass_utils import run_bass_kernel_spmd

    nc = _get_compiled()
    x = np.ascontiguousarray(x, dtype=np.float32)
    wk, wt01, wt2 = _prep_w(route_weights)
    cf, ch = _make_consts()
    in_maps = []
    for ci in range(NCORES):
        xt4, xb = _prep_x_shard(x[ci * BL:(ci + 1) * BL])
        in_maps.append({"wk": wk, "wt01": wt01, "wt2": wt2,
                        "xb": xb, "xt4": xt4, "cf": cf, "ch": ch})
    res = run_bass_kernel_spmd(
        nc, in_maps, list(range(NCORES)),
        trace=bool(int(os.environ.get("CAPS_TRACE", "0"))))
    outs = [_extract(res.results[ci]["o1"], res.results[ci]["o2"])
            for ci in range(NCORES)]
    full = np.concatenate(outs, axis=1)
    if res.exec_time_ns is not None:
        kernel.last_exec_time_ns = res.exec_time_ns
    return full[:, :, None, None, :].astype(np.float32)


kernel.last_exec_time_ns = None
